# revision 7
# baseline (speedup 1.0000x reference)
"""Multi-head attention layer on 8 Trainium2 NeuronCores.

Sharding: 8 cores = 2 (batch) x 4 (head groups of 4 heads).  Each core
computes its batch's attention for its 4 heads plus the partial output
projection (row-parallel); the host sums the 4 partials per batch.

Schedule (single software pipeline, ScalarE exp is the ~147us floor):
  - 11 large input DMAs; first k/q projection chunk starts ~6us in.
  - 8 "windows" (lb, hp) x 16 st-iterations each.  Every iteration emits
    the 2 score matmuls (row-group packed pair) + 1 exp.  The exp stream
    paces the kernel; all other PE work (k/q/v projections, the previous
    window's ctx matmuls, output projections) is interleaved as filler
    between score matmuls so the PE never blocks the ACT stream.
  - ctx accumulation for window w drains during window w+1 (e-tiles are
    buffered in a deep pool), so v-projection does not gate exp start.
  - rowsum reciprocal via reciprocal_approx_fast (5x faster than
    vector.reciprocal), broadcast on GpSimd, scale on DVE.
  - output projection + DMA-out per l-block, interleaved as filler.
"""

import ml_dtypes
import numpy as np

import concourse.bass as bass
import concourse.mybir as mybir
import concourse.tile as tile
from concourse import bacc
from concourse.bass_utils import run_bass_kernel_spmd

F32 = mybir.dt.float32
BF16 = mybir.dt.bfloat16
AF = mybir.ActivationFunctionType
ALU = mybir.AluOpType

P = 128
HD = 64  # head dim

D_MODEL = 1024
N_HEADS = 16
B = 2
L_FULL = 2048
N_CORES = 8
GROUPS = 4  # head groups (tensor parallel)
E_CORE = D_MODEL // GROUPS  # 256 output dims per core for q/k/v


def build_core_kernel(L=2048, D=1024, E=256, LB=512):
    """One core: x[b] attention over E//64 heads. Returns compiled Bacc."""
    assert L % LB == 0 and LB == 512 and D % P == 0 and E % HD == 0
    KT = D // P           # contraction tiles over d_model (8)
    MT_E = E // P         # e tiles (2)
    NLB = L // LB         # l blocks (4)
    ST = L // P           # s tiles (16)
    NH = E // HD          # heads on this core (4)
    HP = NH // 2          # head pairs (2)
    NC_L = LB // P        # l sub-tiles per block (4)
    SCALE = HD ** -0.5
    EA = NH * (HD + 1)    # v columns incl. per-head ones column (260)

    nc = bacc.Bacc("TRN2", target_bir_lowering=False, debug=False)

    xT = nc.declare_dram_parameter("xT", (D, L), BF16, isOutput=False)
    wqT = nc.declare_dram_parameter("wqT", (D, E), BF16, isOutput=False)
    wkT = nc.declare_dram_parameter("wkT", (D, E), BF16, isOutput=False)
    wvT = nc.declare_dram_parameter("wvT", (D, EA), BF16, isOutput=False)
    woT = nc.declare_dram_parameter("woT", (E, D), BF16, isOutput=False)
    bq = nc.declare_dram_parameter("bq", (MT_E, P), F32, isOutput=False)
    bk = nc.declare_dram_parameter("bk", (MT_E, P), F32, isOutput=False)
    bv = nc.declare_dram_parameter("bv", (1, EA), F32, isOutput=False)
    out = nc.declare_dram_parameter("out", (L, D), F32, isOutput=True)

    with tile.TileContext(nc) as tc:
        with (
            tc.tile_pool(name="const", bufs=1) as const,
            tc.tile_pool(name="ps_sc", bufs=2, space="PSUM") as ps_sc,
            tc.tile_pool(name="ps_ctx", bufs=1, space="PSUM") as ps_ctx,
            tc.tile_pool(name="ps_work", bufs=2, space="PSUM") as ps_work,
            tc.tile_pool(name="epool", bufs=22) as epool,
            tc.tile_pool(name="small", bufs=8) as small,
            tc.tile_pool(name="outp", bufs=4) as outp,
        ):
            # ---- resident tensors ----
            xT_sb = const.tile([P, KT, L], BF16)
            wq_sb = const.tile([P, KT, E], BF16)
            wk_sb = const.tile([P, KT, E], BF16)
            wv_sb = const.tile([P, KT, EA], BF16)
            wo_sb = const.tile([P, MT_E, D], BF16)
            qT_sb = const.tile([P, MT_E, L], BF16)
            kT_sb = const.tile([P, MT_E, L], BF16)
            v_sb = const.tile([P, ST, NH, HD + 1], BF16)
            ctxT_sb = const.tile([P, MT_E, L], BF16)
            bq_sb = const.tile([P, MT_E], F32)
            bk_sb = const.tile([P, MT_E], F32)
            bv_row = const.tile([1, EA], F32)
            bv_bc = const.tile([P, EA], F32)

            # ---- input DMAs (few, large) ----
            x_src = xT.rearrange("(o p) l -> p o l", p=P)
            nc.sync.dma_start(xT_sb[:, :, 0:LB], x_src[:, :, 0:LB])
            nc.sync.dma_start(wk_sb[:], wkT.rearrange("(o p) e -> p o e", p=P))
            nc.sync.dma_start(wq_sb[:], wqT.rearrange("(o p) e -> p o e", p=P))
            nc.sync.dma_start(bk_sb[:, :], bk.rearrange("o p -> p o"))
            nc.sync.dma_start(bq_sb[:, :], bq.rearrange("o p -> p o"))
            for c in range(1, NLB):
                nc.sync.dma_start(
                    xT_sb[:, :, c * LB:(c + 1) * LB], x_src[:, :, c * LB:(c + 1) * LB]
                )
            nc.sync.dma_start(wv_sb[:], wvT.rearrange("(o p) e -> p o e", p=P))
            nc.sync.dma_start(bv_row[:, :], bv[:, :])
            nc.sync.dma_start(wo_sb[:], woT.rearrange("(m p) d -> p m d", p=P))
            nc.gpsimd.partition_broadcast(bv_bc[:], bv_row[:])

            # dummy activation: pulls the ~2.7us exp table load to t~0,
            # overlapping the input DMAs instead of the first real exp
            warm = const.tile([1, 16], F32)
            nc.vector.memset(warm[:], 0.0)
            nc.scalar.activation(warm[:], warm[:], AF.Exp)

            # ---- emission helpers (each is a "filler unit" of PE work) ----
            def proj_kq(w_sb, dst, b_sb, m, n):
                """Project one [128, 512] chunk of k or q (8 MMs + bias)."""
                psum = ps_work.tile([P, LB], F32, tag="work", name="pw")
                for kk in range(KT):
                    nc.tensor.matmul(
                        psum[:],
                        lhsT=w_sb[:, kk, m * P:(m + 1) * P],
                        rhs=xT_sb[:, kk, n * LB:(n + 1) * LB],
                        start=(kk == 0),
                        stop=(kk == KT - 1),
                    )
                nc.vector.tensor_scalar_add(
                    dst[:, m, n * LB:(n + 1) * LB], psum[:], b_sb[:, m:m + 1]
                )

            def proj_v(st):
                """Project v for one s-tile (8 MMs + bias)."""
                psum = ps_work.tile([P, LB], F32, tag="work", name="pw")[:, :EA]
                for kk in range(KT):
                    nc.tensor.matmul(
                        psum[:],
                        lhsT=xT_sb[:, kk, st * P:(st + 1) * P],
                        rhs=wv_sb[:, kk, :],
                        start=(kk == 0),
                        stop=(kk == KT - 1),
                    )
                nc.vector.tensor_tensor(
                    v_sb[:, st, :, :],
                    psum[:].rearrange("p (h e) -> p h e", h=NH),
                    bv_bc[:].rearrange("p (h e) -> p h e", h=NH),
                    ALU.add,
                )

            def p3_unit(lb, lt, n):
                """Output projection for one [128, 512] tile + copy to stage."""
                lt_g = lb * NC_L + lt
                psum = ps_work.tile([P, LB], F32, tag="work", name="pw")
                for kk in range(MT_E):
                    nc.tensor.matmul(
                        psum[:],
                        lhsT=ctxT_sb[:, kk, lt_g * P:(lt_g + 1) * P],
                        rhs=wo_sb[:, kk, n * LB:(n + 1) * LB],
                        start=(kk == 0),
                        stop=(kk == MT_E - 1),
                    )
                ot = outp.tile([P, LB], F32, tag="ot")
                nc.vector.tensor_copy(ot[:], psum[:])
                nc.sync.dma_start(
                    out[lt_g * P:(lt_g + 1) * P, n * LB:(n + 1) * LB], ot[:]
                )

            # filler queue of PE work units; emit_units(n) pops n of them
            fq = []

            def emit_units(n):
                for _ in range(n):
                    if not fq:
                        return
                    fq.pop(0)()

            # projection order: k m0 chunks feed window 0 scores (chunk n by
            # iter 4n); q m0 n1..3 feed windows 1..3; v feeds window 0's ctx
            # backlog (drained in window 1, st order); k/q m1 feed hp=1.
            fq.append(lambda: proj_kq(wk_sb, kT_sb, bk_sb, 0, 1))
            fq.append(lambda: proj_v(0))
            fq.append(lambda: proj_kq(wk_sb, kT_sb, bk_sb, 0, 2))
            fq.append(lambda: proj_v(1))
            fq.append(lambda: proj_kq(wk_sb, kT_sb, bk_sb, 0, 3))
            fq.append(lambda: proj_kq(wq_sb, qT_sb, bq_sb, 0, 1))
            fq.append(lambda: proj_v(2))
            fq.append(lambda: proj_v(3))
            fq.append(lambda: proj_kq(wq_sb, qT_sb, bq_sb, 0, 2))
            fq.append(lambda: proj_v(4))
            fq.append(lambda: proj_v(5))
            fq.append(lambda: proj_kq(wq_sb, qT_sb, bq_sb, 0, 3))
            for st in range(6, 10):
                fq.append(lambda st=st: proj_v(st))
            for st in range(10, ST):  # v10..15 drain in window 1 iters 0-5
                fq.append(lambda st=st: proj_v(st))
            for n in range(NLB):
                fq.append(lambda n=n: proj_kq(wk_sb, kT_sb, bk_sb, 1, n))
            for n in range(NLB):
                fq.append(lambda n=n: proj_kq(wq_sb, qT_sb, bq_sb, 1, n))

            # ---- prologue: first k/q chunks so window 0 can start ----
            proj_kq(wk_sb, kT_sb, bk_sb, 0, 0)
            proj_kq(wq_sb, qT_sb, bq_sb, 0, 0)

            # ---- main pipeline ----
            # Window w's ctx matmuls all run during window w+1 (e-tiles are
            # buffered), so exactly one ctx PSUM tile is live at a time.
            windows = [(lb, 0) for lb in range(NLB)] + \
                      [(lb, 1) for lb in range(NLB)]
            NW = len(windows)
            e_tiles = {}       # (w_idx, st) -> e tile
            ctx_of = {}        # w_idx -> ctx psum tile

            def emit_ctx(w_idx, st):
                lb, hp = windows[w_idx]
                if w_idx not in ctx_of:
                    ctx_of[w_idx] = ps_ctx.tile([HD + 1, 2 * LB], F32,
                                                tag="ctx", name="ctxps")
                ctx_ps = ctx_of[w_idx]
                e_t = e_tiles.pop((w_idx, st))
                for hh in range(2):
                    h = 2 * hp + hh
                    nc.tensor.matmul(
                        ctx_ps[:, hh * LB:(hh + 1) * LB],
                        lhsT=v_sb[:, st, h, :],
                        rhs=e_t[:, hh * LB:(hh + 1) * LB],
                        start=(st == 0),
                        stop=(st == ST - 1),
                    )

            recip_of = {}  # w_idx -> [recip tile hh0, recip tile hh1]

            def fctx_step(w_idx, step):
                """One step of the rowsum-normalize chain.  Split so no
                single DVE op blocks the in-order DVE queue long enough to
                stall PE work queued behind it.  Steps 0,1 / 3,4: half-row
                reciprocals; steps 2,5: broadcast + scale (head 0 / 1)."""
                lb, hp = windows[w_idx]
                ctx_ps = ctx_of[w_idx]
                hh = 0 if step < 3 else 1
                sub = step if step < 3 else step - 3
                HL = LB // 2
                if sub == 0:
                    recip_of[w_idx] = recip_of.get(w_idx, [None, None])
                    recip = small.tile([1, LB], F32, tag="recip")
                    recip_of[w_idx][hh] = recip
                if sub < 2:
                    recip = recip_of[w_idx][hh]
                    nc.vector.reciprocal(
                        recip[:, sub * HL:(sub + 1) * HL],
                        ctx_ps[HD:HD + 1, hh * LB + sub * HL:hh * LB + (sub + 1) * HL],
                    )
                else:
                    off = hh * HD
                    recip = recip_of[w_idx][hh]
                    bcast = small.tile([HD, LB], F32, tag="bcast")
                    nc.gpsimd.partition_broadcast(bcast[:], recip[:])
                    nc.vector.tensor_tensor(
                        ctxT_sb[off:off + HD, hp, lb * LB:(lb + 1) * LB],
                        ctx_ps[:HD, hh * LB:(hh + 1) * LB],
                        bcast[:],
                        ALU.mult,
                    )
                    if hh == 1:
                        ctx_of.pop(w_idx)
                        recip_of.pop(w_idx)

            def emit_scores(w_idx, st):
                lb, hp = windows[w_idx]
                sc_ps = ps_sc.tile([P, 2 * LB], F32, tag="sc", name="sc")
                for hh in range(2):
                    off = hh * HD
                    nc.tensor.matmul(
                        sc_ps[:, hh * LB:(hh + 1) * LB],
                        lhsT=kT_sb[off:off + HD, hp, st * P:(st + 1) * P],
                        rhs=qT_sb[off:off + HD, hp, lb * LB:(lb + 1) * LB],
                        start=True,
                        stop=True,
                    )
                e_t = epool.tile([P, 2 * LB], BF16, tag="e", name="e_t")
                nc.scalar.activation(e_t[:], sc_ps[:], AF.Exp, scale=SCALE)
                e_tiles[(w_idx, st)] = e_t

            for w_idx, (lb, hp) in enumerate(windows):
                last_w = w_idx == NW - 1
                drain_rate = 3 if last_w else 2
                for st in range(ST):
                    emit_scores(w_idx, st)
                    if w_idx >= 1:
                        # previous window's ctx accumulation
                        for k2 in range(drain_rate):
                            pst = drain_rate * st + k2
                            if pst < ST:
                                emit_ctx(w_idx - 1, pst)
                        # normalize chain, one step per iteration
                        fs = st - (6 if last_w else 8)
                        if 0 <= fs < 6:
                            fctx_step(w_idx - 1, fs)
                        if fs == 5 and windows[w_idx - 1][1] == 1:
                            plb = windows[w_idx - 1][0]
                            for lt in range(NC_L):
                                for n2 in range(D // LB):
                                    fq.append(lambda plb=plb, lt=lt,
                                              n2=n2: p3_unit(plb, lt, n2))
                        if last_w and st >= 12:
                            for k2 in range(2):
                                emit_ctx(w_idx, 2 * (st - 12) + k2)
                    emit_units(1)

            # ---- tail: last window's remaining ctx + output projection ----
            last = NW - 1
            for st in range(8, ST):
                emit_ctx(last, st)
            for step in range(6):
                fctx_step(last, step)
            emit_units(len(fq))
            lb_last = windows[last][0]
            for lt in range(NC_L):
                for n2 in range(D // LB):
                    p3_unit(lb_last, lt, n2)
    nc.compile()
    return nc


def _augment_wv(wv_slice):
    """Interleave a zero column after each head's 64 value columns."""
    e, d = wv_slice.shape
    nh = e // HD
    aug = np.zeros((nh * (HD + 1), d), dtype=np.float32)
    for h in range(nh):
        aug[h * (HD + 1):h * (HD + 1) + HD] = wv_slice[h * HD:(h + 1) * HD]
    return aug


def _augment_bv(bv_slice):
    """bv with 1.0 in each head's ones-column slot."""
    e = bv_slice.shape[0]
    nh = e // HD
    aug = np.zeros(nh * (HD + 1), dtype=np.float32)
    for h in range(nh):
        aug[h * (HD + 1):h * (HD + 1) + HD] = bv_slice[h * HD:(h + 1) * HD]
        aug[h * (HD + 1) + HD] = 1.0
    return aug


def _core_in_map(core, x, Wq, bq, Wk, bk, Wv, bv, Wo):
    b = core // GROUPS
    g = core % GROUPS
    sl = slice(g * E_CORE, (g + 1) * E_CORE)
    bf = ml_dtypes.bfloat16
    return {
        "xT": np.ascontiguousarray(x[b].T.astype(bf)),
        "wqT": np.ascontiguousarray(Wq[sl, :].T.astype(bf)),
        "wkT": np.ascontiguousarray(Wk[sl, :].T.astype(bf)),
        "wvT": np.ascontiguousarray(_augment_wv(Wv[sl, :]).T.astype(bf)),
        "woT": np.ascontiguousarray(Wo[:, sl].T.astype(bf)),
        "bq": np.ascontiguousarray(bq[sl].reshape(-1, 128)),
        "bk": np.ascontiguousarray(bk[sl].reshape(-1, 128)),
        "bv": np.ascontiguousarray(_augment_bv(bv[sl]).reshape(1, -1)),
    }


_NC_CACHE = {}


def _get_kernel(L, D, E):
    key = (L, D, E)
    if key not in _NC_CACHE:
        _NC_CACHE[key] = build_core_kernel(L=L, D=D, E=E)
    return _NC_CACHE[key]


LAST_RESULT = None


def kernel(x, Wq, bq, Wk, bk, Wv, bv, Wo, bo, trace=False, tmpdir=None):
    x = np.asarray(x, dtype=np.float32)
    Wq = np.asarray(Wq, dtype=np.float32)
    Wk = np.asarray(Wk, dtype=np.float32)
    Wv = np.asarray(Wv, dtype=np.float32)
    Wo = np.asarray(Wo, dtype=np.float32)
    bq = np.asarray(bq, dtype=np.float32)
    bk = np.asarray(bk, dtype=np.float32)
    bv = np.asarray(bv, dtype=np.float32)
    bo = np.asarray(bo, dtype=np.float32)

    Bx, L, D = x.shape
    nc = _get_kernel(L, D, E_CORE)

    in_maps = [
        _core_in_map(core, x, Wq, bq, Wk, bk, Wv, bv, Wo)
        for core in range(N_CORES)
    ]

    global LAST_RESULT
    LAST_RESULT = run_bass_kernel_spmd(
        nc, in_maps, core_ids=list(range(N_CORES)), trace=trace, tmpdir=tmpdir,
    )
    outs = [LAST_RESULT.results[c]["out"] for c in range(N_CORES)]
    full = np.stack(
        [sum(outs[b * GROUPS:(b + 1) * GROUPS]) for b in range(Bx)], axis=0
    )
    return (full + bo).astype(np.float32)


# revision 20
# speedup vs baseline: 1.3352x; 1.3352x over previous
"""Multi-head attention layer on 8 Trainium2 NeuronCores.

Sharding: 8 cores = 2 (batch) x 4 (head groups of 4 heads).  Each core
computes its batch's attention for its 4 heads plus the partial output
projection (row-parallel); the host sums the 4 partials per batch.

Schedule (single software pipeline, ScalarE exp is the ~147us floor):
  - 11 large input DMAs; first k/q projection chunk starts ~6us in.
  - 8 "windows" (lb, hp) x 16 st-iterations each.  Every iteration emits
    the 2 score matmuls (row-group packed pair) + 1 exp.  The exp stream
    paces the kernel; all other PE work (k/q/v projections, the previous
    window's ctx matmuls, output projections) is interleaved as filler
    between score matmuls so the PE never blocks the ACT stream.
  - ctx accumulation for window w drains during window w+1 (e-tiles are
    buffered in a deep pool), so v-projection does not gate exp start.
  - rowsum reciprocal via reciprocal_approx_fast (5x faster than
    vector.reciprocal), broadcast on GpSimd, scale on DVE.
  - output projection + DMA-out per l-block, interleaved as filler.
"""

import ml_dtypes
import numpy as np

import concourse.bass as bass
import concourse.mybir as mybir
import concourse.tile as tile
from concourse import bacc
from concourse.bass_utils import run_bass_kernel_spmd

F32 = mybir.dt.float32
BF16 = mybir.dt.bfloat16
AF = mybir.ActivationFunctionType
ALU = mybir.AluOpType

P = 128
HD = 64  # head dim

D_MODEL = 1024
N_HEADS = 16
B = 2
L_FULL = 2048
N_CORES = 8
GROUPS = 4  # head groups (tensor parallel)
E_CORE = D_MODEL // GROUPS  # 256 output dims per core for q/k/v


def build_core_kernel(L=2048, D=1024, E=256, LB=512):
    """One core: x[b] attention over E//64 heads. Returns compiled Bacc."""
    assert L % LB == 0 and LB == 512 and D % P == 0 and E % HD == 0
    KT = D // P           # contraction tiles over d_model (8)
    MT_E = E // P         # e tiles (2)
    NLB = L // LB         # l blocks (4)
    ST = L // P           # s tiles (16)
    NH = E // HD          # heads on this core (4)
    HP = NH // 2          # head pairs (2)
    NC_L = LB // P        # l sub-tiles per block (4)
    SCALE = HD ** -0.5
    EA = NH * (HD + 1)    # v columns incl. per-head ones column (260)

    nc = bacc.Bacc("TRN2", target_bir_lowering=False, debug=False)

    xT = nc.declare_dram_parameter("xT", (D, L), BF16, isOutput=False)
    wqT = nc.declare_dram_parameter("wqT", (D, E), BF16, isOutput=False)
    wkT = nc.declare_dram_parameter("wkT", (D, E), BF16, isOutput=False)
    wvT = nc.declare_dram_parameter("wvT", (D, EA), BF16, isOutput=False)
    woT = nc.declare_dram_parameter("woT", (E, D), BF16, isOutput=False)
    bq = nc.declare_dram_parameter("bq", (MT_E, P), F32, isOutput=False)
    bk = nc.declare_dram_parameter("bk", (MT_E, P), F32, isOutput=False)
    bv = nc.declare_dram_parameter("bv", (1, EA), F32, isOutput=False)
    out = nc.declare_dram_parameter("out", (L, D), F32, isOutput=True)

    with tile.TileContext(nc) as tc:
        with (
            tc.tile_pool(name="const", bufs=1) as const,
            tc.tile_pool(name="ps_sc", bufs=2, space="PSUM") as ps_sc,
            tc.tile_pool(name="ps_ctx", bufs=1, space="PSUM") as ps_ctx,
            tc.tile_pool(name="ps_work", bufs=2, space="PSUM") as ps_work,
            tc.tile_pool(name="epool", bufs=22) as epool,
            tc.tile_pool(name="small", bufs=8) as small,
            tc.tile_pool(name="outp", bufs=4) as outp,
        ):
            # ---- resident tensors ----
            xT_sb = const.tile([P, KT, L], BF16)
            wq_sb = const.tile([P, KT, E], BF16)
            wk_sb = const.tile([P, KT, E], BF16)
            wv_sb = const.tile([P, KT, EA], BF16)
            wo_sb = const.tile([P, MT_E, D], BF16)
            qT_sb = const.tile([P, MT_E, L], BF16)
            kT_sb = const.tile([P, MT_E, L], BF16)
            v_sb = const.tile([P, ST, NH, HD + 1], BF16)
            ctxT_sb = const.tile([P, MT_E, L], BF16)
            bq_sb = const.tile([P, MT_E], F32)
            bk_sb = const.tile([P, MT_E], F32)
            bv_row = const.tile([1, EA], F32)
            bv_bc = const.tile([P, EA], F32)

            # ---- input DMAs (few, large) ----
            x_src = xT.rearrange("(o p) l -> p o l", p=P)
            nc.sync.dma_start(xT_sb[:, :, 0:LB], x_src[:, :, 0:LB])
            nc.sync.dma_start(wk_sb[:], wkT.rearrange("(o p) e -> p o e", p=P))
            nc.sync.dma_start(wq_sb[:], wqT.rearrange("(o p) e -> p o e", p=P))
            nc.sync.dma_start(bk_sb[:, :], bk.rearrange("o p -> p o"))
            nc.sync.dma_start(bq_sb[:, :], bq.rearrange("o p -> p o"))
            for c in range(1, NLB):
                nc.sync.dma_start(
                    xT_sb[:, :, c * LB:(c + 1) * LB], x_src[:, :, c * LB:(c + 1) * LB]
                )
            nc.sync.dma_start(wv_sb[:], wvT.rearrange("(o p) e -> p o e", p=P))
            nc.sync.dma_start(bv_row[:, :], bv[:, :])
            nc.sync.dma_start(wo_sb[:], woT.rearrange("(m p) d -> p m d", p=P))
            nc.gpsimd.partition_broadcast(bv_bc[:], bv_row[:])

            # dummy activation: pulls the ~2.7us exp table load to t~0,
            # overlapping the input DMAs instead of the first real exp
            warm = const.tile([1, 16], F32)
            nc.vector.memset(warm[:], 0.0)
            nc.scalar.activation(warm[:], warm[:], AF.Exp)

            # persistent scratch for the strided reciprocal result (memset
            # once so the transpose-back never reads uninitialized bytes)
            t2_sc = [const.tile([32, LB], F32, name=f"t2sc{i}")
                     for i in range(2)]
            nc.vector.memset(t2_sc[0][:], 0.0)
            nc.vector.memset(t2_sc[1][:], 0.0)

            # ---- emission helpers (each is a "filler unit" of PE work) ----
            def proj_kq(w_sb, dst, b_sb, m, n):
                """Project one [128, 512] chunk of k or q (8 MMs + bias)."""
                psum = ps_work.tile([P, LB], F32, tag="work", name="pw")
                for kk in range(KT):
                    nc.tensor.matmul(
                        psum[:],
                        lhsT=w_sb[:, kk, m * P:(m + 1) * P],
                        rhs=xT_sb[:, kk, n * LB:(n + 1) * LB],
                        start=(kk == 0),
                        stop=(kk == KT - 1),
                    )
                nc.vector.tensor_scalar_add(
                    dst[:, m, n * LB:(n + 1) * LB], psum[:], b_sb[:, m:m + 1]
                )

            def proj_v(st):
                """Project v for one s-tile (8 MMs + bias)."""
                psum = ps_work.tile([P, LB], F32, tag="work", name="pw")[:, :EA]
                for kk in range(KT):
                    nc.tensor.matmul(
                        psum[:],
                        lhsT=xT_sb[:, kk, st * P:(st + 1) * P],
                        rhs=wv_sb[:, kk, :],
                        start=(kk == 0),
                        stop=(kk == KT - 1),
                    )
                nc.vector.tensor_tensor(
                    v_sb[:, st, :, :],
                    psum[:].rearrange("p (h e) -> p h e", h=NH),
                    bv_bc[:].rearrange("p (h e) -> p h e", h=NH),
                    ALU.add,
                )

            def p3_unit(lb, lt, n):
                """Output projection for one [128, 512] tile + copy to stage."""
                lt_g = lb * NC_L + lt
                psum = ps_work.tile([P, LB], F32, tag="work", name="pw")
                for kk in range(MT_E):
                    nc.tensor.matmul(
                        psum[:],
                        lhsT=ctxT_sb[:, kk, lt_g * P:(lt_g + 1) * P],
                        rhs=wo_sb[:, kk, n * LB:(n + 1) * LB],
                        start=(kk == 0),
                        stop=(kk == MT_E - 1),
                    )
                ot = outp.tile([P, LB], F32, tag="ot")
                nc.vector.tensor_copy(ot[:], psum[:])
                nc.sync.dma_start(
                    out[lt_g * P:(lt_g + 1) * P, n * LB:(n + 1) * LB], ot[:]
                )

            # filler queue of PE work units; emit_units(n) pops n of them
            fq = []

            def emit_units(n):
                for _ in range(n):
                    if not fq:
                        return
                    fq.pop(0)()

            # projection order: k m0 chunks feed window 0 scores (chunk n by
            # iter 4n); q m0 n1..3 feed windows 1..3; v feeds window 0's ctx
            # backlog (drained in window 1, st order); k/q m1 feed hp=1.
            fq.append(lambda: proj_kq(wk_sb, kT_sb, bk_sb, 0, 1))
            fq.append(lambda: proj_v(0))
            fq.append(lambda: proj_kq(wk_sb, kT_sb, bk_sb, 0, 2))
            fq.append(lambda: proj_v(1))
            fq.append(lambda: proj_kq(wk_sb, kT_sb, bk_sb, 0, 3))
            fq.append(lambda: proj_kq(wq_sb, qT_sb, bq_sb, 0, 1))
            fq.append(lambda: proj_v(2))
            fq.append(lambda: proj_v(3))
            fq.append(lambda: proj_kq(wq_sb, qT_sb, bq_sb, 0, 2))
            fq.append(lambda: proj_v(4))
            fq.append(lambda: proj_v(5))
            fq.append(lambda: proj_kq(wq_sb, qT_sb, bq_sb, 0, 3))
            for st in range(6, 10):
                fq.append(lambda st=st: proj_v(st))
            for st in range(10, ST):  # v10..15 drain in window 1 iters 0-5
                fq.append(lambda st=st: proj_v(st))
            for n in range(NLB):
                fq.append(lambda n=n: proj_kq(wk_sb, kT_sb, bk_sb, 1, n))
            for n in range(NLB):
                fq.append(lambda n=n: proj_kq(wq_sb, qT_sb, bq_sb, 1, n))

            # ---- prologue: first k/q chunks so window 0 can start ----
            proj_kq(wk_sb, kT_sb, bk_sb, 0, 0)
            proj_kq(wq_sb, qT_sb, bq_sb, 0, 0)

            # ---- main pipeline ----
            # Window w's ctx matmuls all run during window w+1 (e-tiles are
            # buffered), so exactly one ctx PSUM tile is live at a time.
            windows = [(lb, 0) for lb in range(NLB)] + \
                      [(lb, 1) for lb in range(NLB)]
            NW = len(windows)
            e_tiles = {}       # (w_idx, st) -> e tile
            ctx_of = {}        # w_idx -> ctx psum tile

            def emit_ctx(w_idx, st):
                lb, hp = windows[w_idx]
                if w_idx not in ctx_of:
                    # 96 rows: 0..63 ctx, 64 rowsum, 65..95 memset scratch
                    # so the DVE block-transpose window [64:96) is fully
                    # initialized (the sim rejects uninitialized reads)
                    ctx_of[w_idx] = ps_ctx.tile([96, 2 * LB], F32,
                                                tag="ctx", name="ctxps")
                    # PSUM accesses must start 32-aligned; row 64 is
                    # overwritten right after by the start=True matmul
                    nc.vector.memset(ctx_of[w_idx][HD:96, :], 0.0)
                ctx_ps = ctx_of[w_idx]
                e_t = e_tiles.pop((w_idx, st))
                for hh in range(2):
                    h = 2 * hp + hh
                    nc.tensor.matmul(
                        ctx_ps[:HD + 1, hh * LB:(hh + 1) * LB],
                        lhsT=v_sb[:, st, h, :],
                        rhs=e_t[:, hh * LB:(hh + 1) * LB],
                        start=(st == 0),
                        stop=(st == ST - 1),
                    )

            recip_of = {}  # (w_idx, hh) -> scratch tiles

            def fctx_step(w_idx, step):
                """One step of the rowsum-normalize chain per iteration.
                All DVE ops are <=1us so nothing blocks the in-order DVE
                queue long enough to stall PE work queued behind it.
                Reciprocal runs on a [32,16] layout: the rowsum row is
                block-transposed across partitions (v.transpose works on
                32x32 blocks), inverted, and transposed back.
                Steps per head hh (0: A, 1: B):
                  3*hh+0: transpose rowsum row region + strided reciprocal
                  3*hh+1: transpose back
                  3*hh+2: partition broadcast (GpSimd) + scale TT"""
                lb, hp = windows[w_idx]
                ctx_ps = ctx_of[w_idx]
                hh = step // 3
                sub = step % 3
                if sub == 0:
                    t1 = small.tile([32, LB], F32, tag="t1")
                    nc.vector.transpose(
                        t1[:], ctx_ps[HD:HD + 32, hh * LB:(hh + 1) * LB]
                    )
                    t2 = t2_sc[hh]
                    t1v = t1[:].rearrange("p (b c) -> p b c", c=32)[:, :, 0:1]
                    t2v = t2[:].rearrange("p (b c) -> p b c", c=32)[:, :, 0:1]
                    nc.vector.reciprocal(t2v, t1v)
                    recip_of[(w_idx, hh)] = t2
                elif sub == 1:
                    t2 = recip_of[(w_idx, hh)]
                    t3 = small.tile([32, LB], F32, tag="t3")
                    nc.vector.transpose(t3[:], t2[:])
                    recip_of[(w_idx, hh)] = t3
                else:
                    off = hh * HD
                    t3 = recip_of.pop((w_idx, hh))
                    bcast = small.tile([HD, LB], F32, tag="bcast")
                    nc.gpsimd.partition_broadcast(bcast[:], t3[0:1, :])
                    nc.vector.tensor_tensor(
                        ctxT_sb[off:off + HD, hp, lb * LB:(lb + 1) * LB],
                        ctx_ps[:HD, hh * LB:(hh + 1) * LB],
                        bcast[:],
                        ALU.mult,
                    )
                    if hh == 1:
                        ctx_of.pop(w_idx)

            def emit_scores(w_idx, st):
                lb, hp = windows[w_idx]
                sc_ps = ps_sc.tile([P, 2 * LB], F32, tag="sc", name="sc")
                for hh in range(2):
                    off = hh * HD
                    nc.tensor.matmul(
                        sc_ps[:, hh * LB:(hh + 1) * LB],
                        lhsT=kT_sb[off:off + HD, hp, st * P:(st + 1) * P],
                        rhs=qT_sb[off:off + HD, hp, lb * LB:(lb + 1) * LB],
                        start=True,
                        stop=True,
                    )
                e_t = epool.tile([P, 2 * LB], BF16, tag="e", name="e_t")
                nc.scalar.activation(e_t[:], sc_ps[:], AF.Exp, scale=SCALE)
                e_tiles[(w_idx, st)] = e_t

            for w_idx, (lb, hp) in enumerate(windows):
                last_w = w_idx == NW - 1
                drain_rate = 3 if last_w else 2
                for st in range(ST):
                    emit_scores(w_idx, st)
                    if w_idx >= 1:
                        # previous window's ctx accumulation
                        for k2 in range(drain_rate):
                            pst = drain_rate * st + k2
                            if pst < ST:
                                emit_ctx(w_idx - 1, pst)
                        # normalize chain, one step per iteration
                        fs = st - (6 if last_w else 8)
                        if 0 <= fs < 6:
                            fctx_step(w_idx - 1, fs)
                        if fs == 5 and windows[w_idx - 1][1] == 1:
                            plb = windows[w_idx - 1][0]
                            for lt in range(NC_L):
                                for n2 in range(D // LB):
                                    fq.append(lambda plb=plb, lt=lt,
                                              n2=n2: p3_unit(plb, lt, n2))
                        if last_w and st >= 12:
                            for k2 in range(2):
                                emit_ctx(w_idx, 2 * (st - 12) + k2)
                    emit_units(1)

            # ---- tail: last window's remaining ctx + output projection ----
            last = NW - 1
            for st in range(8, ST):
                emit_ctx(last, st)
            for step in range(6):
                fctx_step(last, step)
            emit_units(len(fq))
            lb_last = windows[last][0]
            for lt in range(NC_L):
                for n2 in range(D // LB):
                    p3_unit(lb_last, lt, n2)
    nc.compile()
    return nc


def _augment_wv(wv_slice):
    """Interleave a zero column after each head's 64 value columns."""
    e, d = wv_slice.shape
    nh = e // HD
    aug = np.zeros((nh * (HD + 1), d), dtype=np.float32)
    for h in range(nh):
        aug[h * (HD + 1):h * (HD + 1) + HD] = wv_slice[h * HD:(h + 1) * HD]
    return aug


def _augment_bv(bv_slice):
    """bv with 1.0 in each head's ones-column slot."""
    e = bv_slice.shape[0]
    nh = e // HD
    aug = np.zeros(nh * (HD + 1), dtype=np.float32)
    for h in range(nh):
        aug[h * (HD + 1):h * (HD + 1) + HD] = bv_slice[h * HD:(h + 1) * HD]
        aug[h * (HD + 1) + HD] = 1.0
    return aug


def _core_in_map(core, x, Wq, bq, Wk, bk, Wv, bv, Wo):
    b = core // GROUPS
    g = core % GROUPS
    sl = slice(g * E_CORE, (g + 1) * E_CORE)
    bf = ml_dtypes.bfloat16
    return {
        "xT": np.ascontiguousarray(x[b].T.astype(bf)),
        "wqT": np.ascontiguousarray(Wq[sl, :].T.astype(bf)),
        "wkT": np.ascontiguousarray(Wk[sl, :].T.astype(bf)),
        "wvT": np.ascontiguousarray(_augment_wv(Wv[sl, :]).T.astype(bf)),
        "woT": np.ascontiguousarray(Wo[:, sl].T.astype(bf)),
        "bq": np.ascontiguousarray(bq[sl].reshape(-1, 128)),
        "bk": np.ascontiguousarray(bk[sl].reshape(-1, 128)),
        "bv": np.ascontiguousarray(_augment_bv(bv[sl]).reshape(1, -1)),
    }


_NC_CACHE = {}


def _get_kernel(L, D, E):
    key = (L, D, E)
    if key not in _NC_CACHE:
        _NC_CACHE[key] = build_core_kernel(L=L, D=D, E=E)
    return _NC_CACHE[key]


LAST_RESULT = None


def kernel(x, Wq, bq, Wk, bk, Wv, bv, Wo, bo, trace=False, tmpdir=None):
    x = np.asarray(x, dtype=np.float32)
    Wq = np.asarray(Wq, dtype=np.float32)
    Wk = np.asarray(Wk, dtype=np.float32)
    Wv = np.asarray(Wv, dtype=np.float32)
    Wo = np.asarray(Wo, dtype=np.float32)
    bq = np.asarray(bq, dtype=np.float32)
    bk = np.asarray(bk, dtype=np.float32)
    bv = np.asarray(bv, dtype=np.float32)
    bo = np.asarray(bo, dtype=np.float32)

    Bx, L, D = x.shape
    nc = _get_kernel(L, D, E_CORE)

    in_maps = [
        _core_in_map(core, x, Wq, bq, Wk, bk, Wv, bv, Wo)
        for core in range(N_CORES)
    ]

    global LAST_RESULT
    LAST_RESULT = run_bass_kernel_spmd(
        nc, in_maps, core_ids=list(range(N_CORES)), trace=trace, tmpdir=tmpdir,
    )
    outs = [LAST_RESULT.results[c]["out"] for c in range(N_CORES)]
    full = np.stack(
        [sum(outs[b * GROUPS:(b + 1) * GROUPS]) for b in range(Bx)], axis=0
    )
    return (full + bo).astype(np.float32)


# revision 28
# speedup vs baseline: 1.3375x; 1.0017x over previous
"""Multi-head attention layer on 8 Trainium2 NeuronCores.

Sharding: 8 cores = 2 (batch) x 4 (head groups of 4 heads).  Each core
computes its batch's attention for its 4 heads plus the partial output
projection (row-parallel); the host sums the 4 partials per batch.

Schedule (single software pipeline, ScalarE exp is the ~147us floor):
  - 11 large input DMAs; first k/q projection chunk starts ~6us in.
  - 8 "windows" (lb, hp) x 16 st-iterations each.  Every iteration emits
    the 2 score matmuls (row-group packed pair) + 1 exp.  The exp stream
    paces the kernel; all other PE work (k/q/v projections, the previous
    window's ctx matmuls, output projections) is interleaved as filler
    between score matmuls so the PE never blocks the ACT stream.
  - ctx accumulation for window w drains during window w+1 (e-tiles are
    buffered in a deep pool), so v-projection does not gate exp start.
  - rowsum reciprocal via reciprocal_approx_fast (5x faster than
    vector.reciprocal), broadcast on GpSimd, scale on DVE.
  - output projection + DMA-out per l-block, interleaved as filler.
"""

import ml_dtypes
import numpy as np

import concourse.bass as bass
import concourse.mybir as mybir
import concourse.tile as tile
from concourse import bacc
from concourse.bass_utils import run_bass_kernel_spmd

F32 = mybir.dt.float32
BF16 = mybir.dt.bfloat16
AF = mybir.ActivationFunctionType
ALU = mybir.AluOpType

P = 128
HD = 64  # head dim

D_MODEL = 1024
N_HEADS = 16
B = 2
L_FULL = 2048
N_CORES = 8
GROUPS = 4  # head groups (tensor parallel)
E_CORE = D_MODEL // GROUPS  # 256 output dims per core for q/k/v


def build_core_kernel(L=2048, D=1024, E=256, LB=512):
    """One core: x[b] attention over E//64 heads. Returns compiled Bacc."""
    assert L % LB == 0 and LB == 512 and D % P == 0 and E % HD == 0
    KT = D // P           # contraction tiles over d_model (8)
    MT_E = E // P         # e tiles (2)
    NLB = L // LB         # l blocks (4)
    ST = L // P           # s tiles (16)
    NH = E // HD          # heads on this core (4)
    HP = NH // 2          # head pairs (2)
    NC_L = LB // P        # l sub-tiles per block (4)
    SCALE = HD ** -0.5
    EA = NH * (HD + 1)    # v columns incl. per-head ones column (260)

    nc = bacc.Bacc("TRN2", target_bir_lowering=False, debug=False)

    # Inputs are pre-shuffled on the host so every DMA reads long
    # contiguous per-partition runs (128 big descriptors per transfer
    # instead of 1000+ small ones -- the input load was descriptor-bound).
    xT = nc.declare_dram_parameter("xT", (NLB * P, KT * LB), BF16,
                                   isOutput=False)
    wqT = nc.declare_dram_parameter("wqT", (P, KT * E), BF16, isOutput=False)
    wkT = nc.declare_dram_parameter("wkT", (P, KT * E), BF16, isOutput=False)
    wvT = nc.declare_dram_parameter("wvT", (P, KT * EA), BF16, isOutput=False)
    woT = nc.declare_dram_parameter("woT", (P, MT_E * D), BF16, isOutput=False)
    bq = nc.declare_dram_parameter("bq", (MT_E, P), F32, isOutput=False)
    bk = nc.declare_dram_parameter("bk", (MT_E, P), F32, isOutput=False)
    bv = nc.declare_dram_parameter("bv", (1, EA), F32, isOutput=False)
    out = nc.declare_dram_parameter("out", (L, D), BF16, isOutput=True)

    with tile.TileContext(nc) as tc:
        with (
            tc.tile_pool(name="const", bufs=1) as const,
            tc.tile_pool(name="ps_sc", bufs=2, space="PSUM") as ps_sc,
            tc.tile_pool(name="ps_ctx", bufs=1, space="PSUM") as ps_ctx,
            tc.tile_pool(name="ps_work", bufs=2, space="PSUM") as ps_work,
            tc.tile_pool(name="epool", bufs=22) as epool,
            tc.tile_pool(name="small", bufs=8) as small,
            tc.tile_pool(name="outp", bufs=4) as outp,
        ):
            # ---- resident tensors ----
            xT_sb = const.tile([P, KT, L], BF16)
            wq_sb = const.tile([P, KT, E], BF16)
            wk_sb = const.tile([P, KT, E], BF16)
            wv_sb = const.tile([P, KT, EA], BF16)
            wo_sb = const.tile([P, MT_E, D], BF16)
            qT_sb = const.tile([P, MT_E, L], BF16)
            kT_sb = const.tile([P, MT_E, L], BF16)
            v_sb = const.tile([P, ST, NH, HD + 1], BF16)
            ctxT_sb = const.tile([P, MT_E, L], BF16)
            bq_sb = const.tile([P, MT_E], F32)
            bk_sb = const.tile([P, MT_E], F32)
            bv_row = const.tile([1, EA], F32)
            bv_bc = const.tile([P, EA], F32)

            # ---- input DMAs (few, large, contiguous per partition) ----
            def x_chunk(c):
                nc.sync.dma_start(
                    xT_sb[:, :, c * LB:(c + 1) * LB],
                    xT[c * P:(c + 1) * P, :].rearrange("p (o l) -> p o l",
                                                       l=LB),
                )

            x_chunk(0)
            nc.sync.dma_start(wk_sb[:], wkT.rearrange("p (o e) -> p o e", e=E))
            nc.sync.dma_start(wq_sb[:], wqT.rearrange("p (o e) -> p o e", e=E))
            nc.sync.dma_start(bk_sb[:, :], bk.rearrange("o p -> p o"))
            nc.sync.dma_start(bq_sb[:, :], bq.rearrange("o p -> p o"))
            for c in range(1, NLB):
                x_chunk(c)
            nc.sync.dma_start(wv_sb[:],
                              wvT.rearrange("p (o e) -> p o e", e=EA))
            nc.sync.dma_start(bv_row[:, :], bv[:, :])
            nc.sync.dma_start(wo_sb[:],
                              woT.rearrange("p (m dd) -> p m dd", dd=D))
            nc.gpsimd.partition_broadcast(bv_bc[:], bv_row[:])

            # PE warm-up: ~4us of dummy matmuls during the DMA wait so the
            # HAM clock gate is at 8/8 before the first real projection
            wsrc = const.tile([P, HD], BF16)
            nc.vector.memset(wsrc[:], 0.0)
            wps = ps_work.tile([HD, HD], F32, tag="work", name="wps")
            for _ in range(20):
                nc.tensor.matmul(wps[:], lhsT=wsrc[:], rhs=wsrc[:, :HD],
                                 start=True, stop=True)

            # dummy activation: pulls the ~2.7us exp table load to t~0,
            # overlapping the input DMAs instead of the first real exp
            warm = const.tile([1, 16], F32)
            nc.vector.memset(warm[:], 0.0)
            nc.scalar.activation(warm[:], warm[:], AF.Exp)

            # persistent scratch for the strided reciprocal result (memset
            # once so the transpose-back never reads uninitialized bytes)
            t2_sc = [const.tile([32, LB], F32, name=f"t2sc{i}")
                     for i in range(2)]
            nc.vector.memset(t2_sc[0][:], 0.0)
            nc.vector.memset(t2_sc[1][:], 0.0)

            # ---- emission helpers (each is a "filler unit" of PE work) ----
            def proj_kq(w_sb, dst, b_sb, m, n):
                """Project one [128, 512] chunk of k or q (8 MMs + bias)."""
                psum = ps_work.tile([P, LB], F32, tag="work", name="pw")
                for kk in range(KT):
                    nc.tensor.matmul(
                        psum[:],
                        lhsT=w_sb[:, kk, m * P:(m + 1) * P],
                        rhs=xT_sb[:, kk, n * LB:(n + 1) * LB],
                        start=(kk == 0),
                        stop=(kk == KT - 1),
                    )
                nc.vector.tensor_scalar_add(
                    dst[:, m, n * LB:(n + 1) * LB], psum[:], b_sb[:, m:m + 1]
                )

            def proj_v(st):
                """Project v for one s-tile (8 MMs + bias)."""
                psum = ps_work.tile([P, LB], F32, tag="work", name="pw")[:, :EA]
                for kk in range(KT):
                    nc.tensor.matmul(
                        psum[:],
                        lhsT=xT_sb[:, kk, st * P:(st + 1) * P],
                        rhs=wv_sb[:, kk, :],
                        start=(kk == 0),
                        stop=(kk == KT - 1),
                    )
                nc.vector.tensor_tensor(
                    v_sb[:, st, :, :],
                    psum[:].rearrange("p (h e) -> p h e", h=NH),
                    bv_bc[:].rearrange("p (h e) -> p h e", h=NH),
                    ALU.add,
                )

            def p3_unit(lb, lt):
                """Output projection for one l-tile: both D halves staged
                into one bf16 tile, written with a single contiguous DMA."""
                lt_g = lb * NC_L + lt
                ot = outp.tile([P, D], BF16, tag="ot")
                for n in range(D // LB):
                    psum = ps_work.tile([P, LB], F32, tag="work", name="pw")
                    for kk in range(MT_E):
                        nc.tensor.matmul(
                            psum[:],
                            lhsT=ctxT_sb[:, kk, lt_g * P:(lt_g + 1) * P],
                            rhs=wo_sb[:, kk, n * LB:(n + 1) * LB],
                            start=(kk == 0),
                            stop=(kk == MT_E - 1),
                        )
                    nc.vector.tensor_copy(ot[:, n * LB:(n + 1) * LB], psum[:])
                nc.sync.dma_start(out[lt_g * P:(lt_g + 1) * P, :], ot[:])

            # filler queue of PE work units; emit_units(n) pops n of them
            fq = []

            def emit_units(n):
                for _ in range(n):
                    if not fq:
                        return
                    fq.pop(0)()

            # projection order: k m0 chunks feed window 0 scores (chunk n by
            # iter 4n); q m0 n1..3 feed windows 1..3; v feeds window 0's ctx
            # backlog (drained in window 1, st order); k/q m1 feed hp=1.
            fq.append(lambda: proj_kq(wk_sb, kT_sb, bk_sb, 0, 1))
            fq.append(lambda: proj_v(0))
            fq.append(lambda: proj_kq(wk_sb, kT_sb, bk_sb, 0, 2))
            fq.append(lambda: proj_v(1))
            fq.append(lambda: proj_kq(wk_sb, kT_sb, bk_sb, 0, 3))
            fq.append(lambda: proj_kq(wq_sb, qT_sb, bq_sb, 0, 1))
            fq.append(lambda: proj_v(2))
            fq.append(lambda: proj_v(3))
            fq.append(lambda: proj_kq(wq_sb, qT_sb, bq_sb, 0, 2))
            fq.append(lambda: proj_v(4))
            fq.append(lambda: proj_v(5))
            fq.append(lambda: proj_kq(wq_sb, qT_sb, bq_sb, 0, 3))
            for st in range(6, 10):
                fq.append(lambda st=st: proj_v(st))
            for st in range(10, ST):  # v10..15 drain in window 1 iters 0-5
                fq.append(lambda st=st: proj_v(st))
            for n in range(NLB):
                fq.append(lambda n=n: proj_kq(wk_sb, kT_sb, bk_sb, 1, n))
            for n in range(NLB):
                fq.append(lambda n=n: proj_kq(wq_sb, qT_sb, bq_sb, 1, n))

            # ---- prologue: first k/q chunks so window 0 can start ----
            proj_kq(wk_sb, kT_sb, bk_sb, 0, 0)
            proj_kq(wq_sb, qT_sb, bq_sb, 0, 0)

            # ---- main pipeline ----
            # Window w's ctx matmuls all run during window w+1 (e-tiles are
            # buffered), so exactly one ctx PSUM tile is live at a time.
            windows = [(lb, 0) for lb in range(NLB)] + \
                      [(lb, 1) for lb in range(NLB)]
            NW = len(windows)
            e_tiles = {}       # (w_idx, st) -> e tile
            ctx_of = {}        # w_idx -> ctx psum tile

            def emit_ctx(w_idx, st):
                lb, hp = windows[w_idx]
                if w_idx not in ctx_of:
                    # 96 rows: 0..63 ctx, 64 rowsum, 65..95 memset scratch
                    # so the DVE block-transpose window [64:96) is fully
                    # initialized (the sim rejects uninitialized reads)
                    ctx_of[w_idx] = ps_ctx.tile([96, 2 * LB], F32,
                                                tag="ctx", name="ctxps")
                    # PSUM accesses must start 32-aligned; row 64 is
                    # overwritten right after by the start=True matmul
                    nc.vector.memset(ctx_of[w_idx][HD:96, :], 0.0)
                ctx_ps = ctx_of[w_idx]
                e_t = e_tiles.pop((w_idx, st))
                for hh in range(2):
                    h = 2 * hp + hh
                    nc.tensor.matmul(
                        ctx_ps[:HD + 1, hh * LB:(hh + 1) * LB],
                        lhsT=v_sb[:, st, h, :],
                        rhs=e_t[:, hh * LB:(hh + 1) * LB],
                        start=(st == 0),
                        stop=(st == ST - 1),
                    )

            recip_of = {}  # (w_idx, hh) -> scratch tiles

            def fctx_step(w_idx, step):
                """One step of the rowsum-normalize chain per iteration.
                All DVE ops are <=1us so nothing blocks the in-order DVE
                queue long enough to stall PE work queued behind it.
                Reciprocal runs on a [32,16] layout: the rowsum row is
                block-transposed across partitions (v.transpose works on
                32x32 blocks), inverted, and transposed back.
                Steps per head hh (0: A, 1: B):
                  3*hh+0: transpose rowsum row region + strided reciprocal
                  3*hh+1: transpose back
                  3*hh+2: partition broadcast (GpSimd) + scale TT"""
                lb, hp = windows[w_idx]
                ctx_ps = ctx_of[w_idx]
                hh = step // 3
                sub = step % 3
                if sub == 0:
                    t1 = small.tile([32, LB], F32, tag="t1")
                    nc.vector.transpose(
                        t1[:], ctx_ps[HD:HD + 32, hh * LB:(hh + 1) * LB]
                    )
                    t2 = t2_sc[hh]
                    t1v = t1[:].rearrange("p (b c) -> p b c", c=32)[:, :, 0:1]
                    t2v = t2[:].rearrange("p (b c) -> p b c", c=32)[:, :, 0:1]
                    nc.vector.reciprocal(t2v, t1v)
                    recip_of[(w_idx, hh)] = t2
                elif sub == 1:
                    t2 = recip_of[(w_idx, hh)]
                    t3 = small.tile([32, LB], F32, tag="t3")
                    nc.vector.transpose(t3[:], t2[:])
                    recip_of[(w_idx, hh)] = t3
                else:
                    off = hh * HD
                    t3 = recip_of.pop((w_idx, hh))
                    bcast = small.tile([HD, LB], F32, tag="bcast")
                    nc.gpsimd.partition_broadcast(bcast[:], t3[0:1, :])
                    nc.vector.tensor_tensor(
                        ctxT_sb[off:off + HD, hp, lb * LB:(lb + 1) * LB],
                        ctx_ps[:HD, hh * LB:(hh + 1) * LB],
                        bcast[:],
                        ALU.mult,
                    )
                    if hh == 1:
                        ctx_of.pop(w_idx)

            def emit_scores(w_idx, st):
                lb, hp = windows[w_idx]
                sc_ps = ps_sc.tile([P, 2 * LB], F32, tag="sc", name="sc")
                for hh in range(2):
                    off = hh * HD
                    nc.tensor.matmul(
                        sc_ps[:, hh * LB:(hh + 1) * LB],
                        lhsT=kT_sb[off:off + HD, hp, st * P:(st + 1) * P],
                        rhs=qT_sb[off:off + HD, hp, lb * LB:(lb + 1) * LB],
                        start=True,
                        stop=True,
                    )
                e_t = epool.tile([P, 2 * LB], BF16, tag="e", name="e_t")
                nc.scalar.activation(e_t[:], sc_ps[:], AF.Exp, scale=SCALE)
                e_tiles[(w_idx, st)] = e_t

            for w_idx, (lb, hp) in enumerate(windows):
                last_w = w_idx == NW - 1
                drain_rate = 3 if last_w else 2
                for st in range(ST):
                    emit_scores(w_idx, st)
                    if w_idx >= 1:
                        # previous window's ctx accumulation
                        for k2 in range(drain_rate):
                            pst = drain_rate * st + k2
                            if pst < ST:
                                emit_ctx(w_idx - 1, pst)
                        # normalize chain, one step per iteration
                        fs = st - (6 if last_w else 8)
                        if 0 <= fs < 6:
                            fctx_step(w_idx - 1, fs)
                        if fs == 5 and windows[w_idx - 1][1] == 1:
                            plb = windows[w_idx - 1][0]
                            for lt in range(NC_L):
                                fq.append(lambda plb=plb, lt=lt:
                                          p3_unit(plb, lt))
                        if last_w and st >= 12:
                            for k2 in range(2):
                                emit_ctx(w_idx, 2 * (st - 12) + k2)
                    emit_units(1)

            # ---- tail: last window's remaining ctx + output projection ----
            last = NW - 1
            for st in range(8, ST):
                emit_ctx(last, st)
            for step in range(6):
                fctx_step(last, step)
            emit_units(len(fq))
            lb_last = windows[last][0]
            for lt in range(NC_L):
                p3_unit(lb_last, lt)
    nc.compile()
    return nc


def _augment_wv(wv_slice):
    """Interleave a zero column after each head's 64 value columns."""
    e, d = wv_slice.shape
    nh = e // HD
    aug = np.zeros((nh * (HD + 1), d), dtype=np.float32)
    for h in range(nh):
        aug[h * (HD + 1):h * (HD + 1) + HD] = wv_slice[h * HD:(h + 1) * HD]
    return aug


def _augment_bv(bv_slice):
    """bv with 1.0 in each head's ones-column slot."""
    e = bv_slice.shape[0]
    nh = e // HD
    aug = np.zeros(nh * (HD + 1), dtype=np.float32)
    for h in range(nh):
        aug[h * (HD + 1):h * (HD + 1) + HD] = bv_slice[h * HD:(h + 1) * HD]
        aug[h * (HD + 1) + HD] = 1.0
    return aug


def _shuffle_x(xT):
    """[D, L] -> [(c p), (o l)]: per-partition-contiguous chunked layout."""
    D, L = xT.shape
    a = xT.reshape(D // 128, 128, L // 512, 512)      # [o, p, c, l]
    return a.transpose(2, 1, 0, 3).reshape(L // 512 * 128, D // 128 * 512)


def _shuffle_w(wT):
    """[D, E] -> [p, (o e)]: per-partition-contiguous weight layout."""
    D, E = wT.shape
    a = wT.reshape(D // 128, 128, E)                  # [o, p, e]
    return a.transpose(1, 0, 2).reshape(128, D // 128 * E)


def _core_in_map(core, x, Wq, bq, Wk, bk, Wv, bv, Wo):
    b = core // GROUPS
    g = core % GROUPS
    sl = slice(g * E_CORE, (g + 1) * E_CORE)
    bf = ml_dtypes.bfloat16
    return {
        "xT": np.ascontiguousarray(_shuffle_x(x[b].T.astype(bf))),
        "wqT": np.ascontiguousarray(_shuffle_w(Wq[sl, :].T.astype(bf))),
        "wkT": np.ascontiguousarray(_shuffle_w(Wk[sl, :].T.astype(bf))),
        "wvT": np.ascontiguousarray(
            _shuffle_w(_augment_wv(Wv[sl, :]).T.astype(bf))),
        "woT": np.ascontiguousarray(_shuffle_w(Wo[:, sl].T.astype(bf))),
        "bq": np.ascontiguousarray(bq[sl].reshape(-1, 128)),
        "bk": np.ascontiguousarray(bk[sl].reshape(-1, 128)),
        "bv": np.ascontiguousarray(_augment_bv(bv[sl]).reshape(1, -1)),
    }


_NC_CACHE = {}


def _get_kernel(L, D, E):
    key = (L, D, E)
    if key not in _NC_CACHE:
        _NC_CACHE[key] = build_core_kernel(L=L, D=D, E=E)
    return _NC_CACHE[key]


LAST_RESULT = None


def kernel(x, Wq, bq, Wk, bk, Wv, bv, Wo, bo, trace=False, tmpdir=None):
    x = np.asarray(x, dtype=np.float32)
    Wq = np.asarray(Wq, dtype=np.float32)
    Wk = np.asarray(Wk, dtype=np.float32)
    Wv = np.asarray(Wv, dtype=np.float32)
    Wo = np.asarray(Wo, dtype=np.float32)
    bq = np.asarray(bq, dtype=np.float32)
    bk = np.asarray(bk, dtype=np.float32)
    bv = np.asarray(bv, dtype=np.float32)
    bo = np.asarray(bo, dtype=np.float32)

    Bx, L, D = x.shape
    nc = _get_kernel(L, D, E_CORE)

    in_maps = [
        _core_in_map(core, x, Wq, bq, Wk, bk, Wv, bv, Wo)
        for core in range(N_CORES)
    ]

    global LAST_RESULT
    LAST_RESULT = run_bass_kernel_spmd(
        nc, in_maps, core_ids=list(range(N_CORES)), trace=trace, tmpdir=tmpdir,
    )
    outs = [np.asarray(LAST_RESULT.results[c]["out"], dtype=np.float32)
            for c in range(N_CORES)]
    full = np.stack(
        [sum(outs[b * GROUPS:(b + 1) * GROUPS]) for b in range(Bx)], axis=0
    )
    return (full + bo).astype(np.float32)
